# revision 1
# baseline (speedup 1.0000x reference)
"""Chamfer-loss-overlap kernel for 8 Trainium2 NeuronCores.

Math (per batch element, reference semantics):
    P[i,j] = |x_i|^2 + |y_j|^2 - 2 x_i . y_j          (4096 x 4096)
    a = mean(x_mask * min_i P[i,j])    (min over i, per y-point j)
    b = mean(y_mask * min_j P[i,j])    (min over j, per x-point i)
    out = (a - b)^2

Sharding: batch dim B=8 across the 8 cores (data parallel). Each core
computes its own 4096x4096 distance matrix twice (once per min direction,
so both reductions are along the free dim) and returns the two 4096-long
min vectors. Host applies masks / means in float64 and squares the
difference (scalar combine; the all-reduce of two scalars is trivial).

Device kernel strategy:
  - The distance matrix is produced on TensorE as ONE K=13 bf16 matmul per
    128x512 tile: fp32 x/y are split hi/lo into bf16 (x ~ xh + xl), and the
    |x|^2 / |y|^2 terms ride along as extra contraction rows against ones.
    The per-element error of this split is ~1e-6 relative to P values.
  - Row-mins: ScalarE casts each 2-bank PSUM strip to bf16 in SBUF
    (2x+ cast rate); VectorE reduce-min consumes the 16-bit copy at its
    2x bf16 mode, with 4 strips in flight. Final fold + DMA of the two
    [128, 32] min tensors; masked means run on the host in float64.
"""

import numpy as np
from ml_dtypes import bfloat16

import concourse.bacc as bacc
import concourse.bass as bass
import concourse.mybir as mybir
from concourse import tile

B, N, D = 8, 4096, 3
NCORES = 8
NT = N // 128        # 32 output tiles per pass
QW = 512             # one PSUM bank of fp32
K = 13               # contraction rows of the augmented matmul

# PSUM strip geometry: SW-wide strips, PSUM_BUFS in flight
# (SW * PSUM_BUFS * 4B must be <= 16KB per partition = 8 banks)
SW = 1024
PSUM_BUFS = 4
NSTRIP = N // SW
CPY_BUFS = 12

# Reduction strategy:
#   "cast16b":  ScalarE casts each PSUM strip to bf16 in SBUF (fast 2x+
#               cast), VectorE reduce-min runs at 2x on bf16 (fastest,
#               ~4e-3 extra rel err from bf16 min values)
#   "cast16":   same but fp16 (better precision; ACT cast and reduce both
#               drop to 1x -> slower)
#   "red_psum": VectorE reduce-min directly from PSUM at 1x (safe fallback)
# (tensor_tensor_reduce faults TRN2 hardware on this path - do not use)
REDUCE_MODE = "cast16b"

# Pack the 4 matmuls of each PSUM strip into the 4 32-row groups of the PE
# array (K=13 fits in one group), so they run concurrently (~3x TensorE).
ROW_PACK = False

_CACHE = {}


def _build_nc(reps=1):
    dt = mybir.dt
    nc = bacc.Bacc("TRN2", target_bir_lowering=False, debug=False,
                   num_devices=NCORES)

    la_d = nc.dram_tensor("la", [K, N], dt.bfloat16, kind="ExternalInput")
    ra_d = nc.dram_tensor("ra", [K, N], dt.bfloat16, kind="ExternalInput")
    lb_d = nc.dram_tensor("lb", [K, N], dt.bfloat16, kind="ExternalInput")
    rb_d = nc.dram_tensor("rb", [K, N], dt.bfloat16, kind="ExternalInput")
    minsA_d = nc.dram_tensor("minsA", [128, NT], dt.float32,
                             kind="ExternalOutput")
    minsB_d = nc.dram_tensor("minsB", [128, NT], dt.float32,
                             kind="ExternalOutput")

    with tile.TileContext(nc) as tc:
        with (
            tc.tile_pool(name="rows", bufs=1) as rows,
            tc.tile_pool(name="accs", bufs=1) as accs,
        ):
            npart = 96 + K if ROW_PACK else K
            la = rows.tile([npart, N], dt.bfloat16, tag="la")
            ra = rows.tile([npart, N], dt.bfloat16, tag="ra")
            lb = rows.tile([npart, N], dt.bfloat16, tag="lb")
            rb = rows.tile([npart, N], dt.bfloat16, tag="rb")
            for t, d in ((la, la_d), (ra, ra_d), (lb, lb_d), (rb, rb_d)):
                if ROW_PACK:
                    for r in range(4):
                        nc.sync.dma_start(t[32 * r:32 * r + K, :], d[:])
                else:
                    nc.sync.dma_start(t[:], d[:])

            acc_d = NSTRIP
            accA = accs.tile([128, NT, acc_d], dt.float32, tag="accA")
            accB = accs.tile([128, NT, acc_d], dt.float32, tag="accB")

            import contextlib
            rep_ctx = (tc.For_i(0, reps, 1) if reps > 1
                       else contextlib.nullcontext())
            with rep_ctx:
                _emit_main(nc, tc, la, ra, lb, rb, accA, accB)

            finA = accs.tile([128, NT], dt.float32, tag="finA")
            finB = accs.tile([128, NT], dt.float32, tag="finB")
            nc.vector.tensor_reduce(finA[:], accA[:],
                                    axis=mybir.AxisListType.X,
                                    op=mybir.AluOpType.min)
            nc.vector.tensor_reduce(finB[:], accB[:],
                                    axis=mybir.AxisListType.X,
                                    op=mybir.AluOpType.min)
            nc.sync.dma_start(minsA_d[:], finA[:])
            nc.sync.dma_start(minsB_d[:], finB[:])

    nc.compile()
    return nc


def _emit_main(nc, tc, la, ra, lb, rb, accA, accB):
    dt = mybir.dt
    with (
        tc.tile_pool(name="psum", bufs=PSUM_BUFS,
                     space=bass.MemorySpace.PSUM) as psum,
        tc.tile_pool(name="cpy", bufs=CPY_BUFS) as cpy,
    ):
        cdt = dt.bfloat16 if REDUCE_MODE == "cast16b" else dt.float16
        for L, R, acc in ((la, ra, accA), (lb, rb, accB)):
            for it in range(NT):
                i0 = it * 128
                for h in range(NSTRIP):
                    ps = psum.tile([128, SW], dt.float32, tag="ps", name="ps")
                    for q in range(SW // QW):
                        j0 = h * SW + q * QW
                        nc.tensor.matmul(
                            ps[:, q * QW:(q + 1) * QW],
                            L[:, i0:i0 + 128],
                            R[:, j0:j0 + QW],
                            start=True, stop=True,
                        )
                    if REDUCE_MODE in ("cast16", "cast16b"):
                        cp = cpy.tile([128, SW], cdt, tag="cp", name="cp")
                        nc.scalar.copy(cp[:], ps[:, :])
                        nc.vector.tensor_reduce(
                            acc[:, it, h:h + 1],
                            cp[:],
                            axis=mybir.AxisListType.X,
                            op=mybir.AluOpType.min,
                        )
                    else:
                        nc.vector.tensor_reduce(
                            acc[:, it, h:h + 1],
                            ps[:, :],
                            axis=mybir.AxisListType.X,
                            op=mybir.AluOpType.min,
                        )


def get_nc():
    if "nc" not in _CACHE:
        _CACHE["nc"] = _build_nc()
    return _CACHE["nc"]


def _make_runner(nc):
    """Build a cached jitted SPMD callable for `nc` (one NEFF on all 8
    cores, per-core inputs sharded along axis 0). Mirrors
    bass2jax.run_bass_via_pjrt's multi-core path, but reusable across
    calls so jax tracing/lowering happens once."""
    import jax
    from jax.sharding import Mesh, PartitionSpec
    from jax.experimental.shard_map import shard_map
    from concourse.bass2jax import (
        _bass_exec_p,
        install_neuronx_cc_hook,
        partition_id_tensor,
    )

    install_neuronx_cc_hook()
    partition_name = (nc.partition_id_tensor.name
                      if nc.partition_id_tensor else None)

    in_names = []
    out_names = []
    out_avals = []
    out_shapes = []
    for alloc in nc.m.functions[0].allocations:
        if not isinstance(alloc, mybir.MemoryLocationSet):
            continue
        name = alloc.memorylocations[0].name
        if alloc.kind == "ExternalInput":
            if name != partition_name:
                in_names.append(name)
        elif alloc.kind == "ExternalOutput":
            shape = tuple(alloc.tensor_shape)
            dtype = mybir.dt.np(alloc.dtype)
            out_avals.append(jax.core.ShapedArray(shape, dtype))
            out_names.append(name)
            out_shapes.append((shape, dtype))
    n_params = len(in_names)
    n_outs = len(out_names)
    all_names = list(in_names) + list(out_names)
    if partition_name is not None:
        all_names.append(partition_name)
    donate = tuple(range(n_params, n_params + n_outs))

    def _body(*args):
        operands = list(args)
        if partition_name is not None:
            operands.append(partition_id_tensor())
        outs = _bass_exec_p.bind(
            *operands,
            out_avals=tuple(out_avals),
            in_names=tuple(all_names),
            out_names=tuple(out_names),
            lowering_input_output_aliases=(),
            sim_require_finite=True,
            sim_require_nnan=True,
            nc=nc,
        )
        return tuple(outs)

    devices = jax.devices()[:NCORES]
    mesh = Mesh(np.asarray(devices), ("core",))
    sharded = jax.jit(
        shard_map(_body, mesh=mesh,
                  in_specs=(PartitionSpec("core"),) * (n_params + n_outs),
                  out_specs=(PartitionSpec("core"),) * n_outs,
                  check_rep=False),
        donate_argnums=donate,
        keep_unused=True,
    )

    def prep(in_maps):
        concat_in = [
            np.concatenate([np.asarray(m[name]) for m in in_maps], axis=0)
            for name in in_names
        ]
        return concat_in

    def exec_prepped(concat_in):
        concat_zeros = [
            np.zeros((NCORES * s[0], *s[1:]), dt) for s, dt in out_shapes
        ]
        return sharded(*concat_in, *concat_zeros)

    def unpack(out_arrs):
        return [
            {
                name: np.asarray(out_arrs[i]).reshape(
                    NCORES, *out_shapes[i][0])[c]
                for i, name in enumerate(out_names)
            }
            for c in range(NCORES)
        ]

    def run(in_maps):
        return unpack(exec_prepped(prep(in_maps)))

    run.prep = prep
    run.exec_prepped = exec_prepped
    run.unpack = unpack
    run.mesh = mesh
    return run


def get_runner():
    if "run" not in _CACHE:
        _CACHE["run"] = _make_runner(get_nc())
    return _CACHE["run"]


def _f32(v):
    return np.asarray(v, dtype=np.float32)


def _bf(v):
    return np.asarray(v, dtype=np.float32).astype(bfloat16)


def build_rows(xc, yc):
    """Build the four [13, 4096] bf16 row tensors for one batch element.

    Contraction layout (k : L-row      * R-row):
      0-2 : -2*xh_d  * yh_d
      3-5 : -2*xl_d  * yh_d
      6-8 : -2*xh_d  * yl_d
      9   : sqx_h    * 1
      10  : sqx_l    * 1
      11  : 1        * sqy_h
      12  : 1        * sqy_l
    Pass B swaps the roles of x and y with the identical term multiset, so
    P_B = P_A^T up to fp32 accumulation order.
    """
    def side(v):
        vh = _bf(v)
        vl = _bf(_f32(v) - _f32(vh))
        sq = (np.asarray(v, np.float64) ** 2).sum(-1)
        sqh = _bf(sq)
        sql = _bf(sq - np.float64(1.0) * _f32(sqh).astype(np.float64))
        m2h = _bf(-2.0 * _f32(vh))
        m2l = _bf(-2.0 * _f32(vl))
        return vh, vl, sqh, sql, m2h, m2l

    xh, xl, sqxh, sqxl, m2xh, m2xl = side(xc)
    yh, yl, sqyh, sqyl, m2yh, m2yl = side(yc)
    ones = np.ones((N,), dtype=bfloat16)

    def lrows(m2h, m2l, sqh, sql):
        return np.stack([m2h[:, 0], m2h[:, 1], m2h[:, 2],
                         m2l[:, 0], m2l[:, 1], m2l[:, 2],
                         m2h[:, 0], m2h[:, 1], m2h[:, 2],
                         sqh, sql, ones, ones])

    def rrows(vh, vl, sqh, sql):
        return np.stack([vh[:, 0], vh[:, 1], vh[:, 2],
                         vh[:, 0], vh[:, 1], vh[:, 2],
                         vl[:, 0], vl[:, 1], vl[:, 2],
                         ones, ones, sqh, sql])

    return {
        "la": np.ascontiguousarray(lrows(m2xh, m2xl, sqxh, sqxl)),
        "ra": np.ascontiguousarray(rrows(yh, yl, sqyh, sqyl)),
        "lb": np.ascontiguousarray(lrows(m2yh, m2yl, sqyh, sqyl)),
        "rb": np.ascontiguousarray(rrows(xh, xl, sqxh, sqxl)),
    }


def _mins_to_vec(m):
    # m[p, it] is the min for point index it*128 + p
    return np.asarray(m, np.float64).T.reshape(N)


def kernel(x, y, x_mask, y_mask):
    x = np.asarray(x)
    y = np.asarray(y)
    in_maps = [build_rows(x[c], y[c]) for c in range(B)]
    res = get_runner()(in_maps)

    sa = 0.0
    sb = 0.0
    for c in range(B):
        minsA = _mins_to_vec(res[c]["minsA"])  # min over j, per x-point i
        minsB = _mins_to_vec(res[c]["minsB"])  # min over i, per y-point j
        sa += (np.asarray(x_mask[c], np.float64) * minsB).sum()
        sb += (np.asarray(y_mask[c], np.float64) * minsA).sum()
    a = sa / (B * N)
    b = sb / (B * N)
    return np.asarray((a - b) ** 2, dtype=np.float32)



# revision 2
# speedup vs baseline: 4.1731x; 4.1731x over previous
"""Chamfer-loss-overlap kernel for 8 Trainium2 NeuronCores.

Math (per batch element, reference semantics):
    P[i,j] = |x_i|^2 + |y_j|^2 - 2 x_i . y_j          (4096 x 4096)
    a = mean(x_mask * min_i P[i,j])    (min over i, per y-point j)
    b = mean(y_mask * min_j P[i,j])    (min over j, per x-point i)
    out = (a - b)^2

Sharding: batch dim B=8 across the 8 cores (data parallel). Each core
computes its 4096x4096 distance matrix ONCE and extracts both min
directions from the same PSUM tiles:
  - row-min (min over j per x-point i): free-dim reduce of the bf16 copy
  - col-min (min over i per y-point j): running elementwise bf16 min
    across the 32 row-tiles, then a TensorE-transpose + reduce tail to
    fold the remaining 128 partitions.
Host applies masks / means in float64 and squares the difference.

Device kernel strategy:
  - The distance matrix is produced on TensorE as ONE K=13 bf16 matmul per
    128x512 tile: fp32 x/y are split hi/lo into bf16 (x ~ xh + xl), and the
    |x|^2 / |y|^2 terms ride along as extra contraction rows against ones.
  - ScalarE casts each PSUM strip to bf16 in SBUF; VectorE does the
    row-min (fold + reduce); the col-min accumulation (tensor_tensor min
    at the DVE 2x bf16 rate) runs on VectorE and/or GpSimd.
  - bf16 rounding is monotone, so min(bf16(P)) == bf16(min(P)): both
    reductions see identically-rounded values.
"""

import numpy as np
from ml_dtypes import bfloat16

import concourse.bacc as bacc
import concourse.bass as bass
import concourse.mybir as mybir
from concourse import tile
from concourse import masks

B, N, D = 8, 4096, 3
NCORES = 8
NT = N // 128        # 32 row-tiles
QW = 512             # one PSUM bank of fp32 (max matmul free dim)
K = 13               # contraction rows of the augmented matmul

SW = 1024            # PSUM strip width (2 banks)
NSTRIP = N // SW     # 4 strips per row-tile
PSUM_BUFS = 4        # 4 x 2 banks = all 8 banks
CPY_BUFS = 3         # [128, N] bf16 staging buffers

# --- tuning flags -----------------------------------------------------
# NBANDS: 1 = plain 128x128 PE, 4 = pack K=13 matmuls into the 4 32-row
#         PE bands (tile_position) so they stream concurrently.
NBANDS = 1
# COL_ENGINE: engine(s) for the running col-min accumulation.
#   "vector" | "gpsimd" | "split" (gpsimd takes GP_QUARTERS quarters)
COL_ENGINE = "vector"
GP_QUARTERS = 3
# ROW_MODE: "fold" = 3x tensor_tensor min then reduce on [128,1024]
#           "direct" = single tensor_reduce over [128,4096]
ROW_MODE = "fold"
# ---------------------------------------------------------------------

_CACHE = {}


def _build_nc(reps=1):
    dt = mybir.dt
    amin = mybir.AluOpType.min
    nc = bacc.Bacc("TRN2", target_bir_lowering=False, debug=False,
                   num_devices=NCORES)

    la_d = nc.dram_tensor("la", [K, N], dt.bfloat16, kind="ExternalInput")
    ra_d = nc.dram_tensor("ra", [K, N], dt.bfloat16, kind="ExternalInput")
    minsA_d = nc.dram_tensor("minsA", [128, NT], dt.float32,
                             kind="ExternalOutput")
    minsB_d = nc.dram_tensor("minsB", [128, NT], dt.float32,
                             kind="ExternalOutput")

    with tile.TileContext(nc) as tc:
        with (
            tc.tile_pool(name="rows", bufs=1) as rows,
            tc.tile_pool(name="accs", bufs=1) as accs,
        ):
            npart = 96 + K if NBANDS == 4 else K
            la = rows.tile([npart, N], dt.bfloat16, tag="la")
            ra = rows.tile([npart, N], dt.bfloat16, tag="ra")
            for t, d in ((la, la_d), (ra, ra_d)):
                if NBANDS == 4:
                    for r in range(4):
                        nc.sync.dma_start(t[32 * r:32 * r + K, :], d[:])
                else:
                    nc.sync.dma_start(t[:], d[:])

            ident = rows.tile([128, 128], dt.bfloat16, tag="ident")
            masks.make_identity(nc, ident[:])

            colmin = accs.tile([128, N], dt.bfloat16, tag="colmin")
            rowacc = accs.tile([128, NT], dt.float32, tag="rowacc")

            import contextlib
            rep_ctx = (tc.For_i(0, reps, 1) if reps > 1
                       else contextlib.nullcontext())
            with rep_ctx:
                _emit_main(nc, tc, la, ra, colmin, rowacc)

            # tail: fold colmin's 128 partitions per column via TensorE
            # transpose (bf16, 128x128 blocks) + free-dim reduce from PSUM
            minsB_sb = accs.tile([128, NT], dt.float32, tag="minsB_sb")
            with tc.tile_pool(name="tpsum", bufs=2,
                              space=bass.MemorySpace.PSUM) as tpsum:
                for g in range(NT // 4):
                    pst = tpsum.tile([128, 4, 128], dt.bfloat16, tag="pst")
                    for b4 in range(4):
                        t = g * 4 + b4
                        nc.tensor.transpose(
                            pst[:, b4, :],
                            colmin[:, t * 128:(t + 1) * 128],
                            ident[:],
                        )
                    nc.vector.tensor_reduce(minsB_sb[:, g * 4:(g + 1) * 4],
                                            pst[:],
                                            axis=mybir.AxisListType.X,
                                            op=amin)
            nc.sync.dma_start(minsA_d[:], rowacc[:])
            nc.sync.dma_start(minsB_d[:], minsB_sb[:])

    nc.compile()
    return nc


def _emit_main(nc, tc, la, ra, colmin, rowacc):
    dt = mybir.dt
    amin = mybir.AluOpType.min
    with (
        tc.tile_pool(name="psum", bufs=PSUM_BUFS,
                     space=bass.MemorySpace.PSUM) as psum,
        tc.tile_pool(name="cpy", bufs=CPY_BUFS) as cpy,
        tc.tile_pool(name="tmps", bufs=6) as tmps,
    ):
        for it in range(NT):
            i0 = it * 128
            cp = cpy.tile([128, N], dt.bfloat16, tag="cp", name="cp")
            for h in range(NSTRIP):
                bo = 32 * (h % NBANDS)
                ps = psum.tile([128, SW], dt.float32, tag="ps", name="ps")
                for q in range(SW // QW):
                    j0 = h * SW + q * QW
                    nc.tensor.matmul(
                        ps[:, q * QW:(q + 1) * QW],
                        la[bo:bo + K, i0:i0 + 128],
                        ra[bo:bo + K, j0:j0 + QW],
                        start=True, stop=True,
                        tile_position=(bo, 0) if NBANDS > 1 else None,
                    )
                nc.scalar.copy(cp[:, h * SW:(h + 1) * SW], ps[:])

            # running col-min (elementwise, bf16 2x)
            if COL_ENGINE == "split":
                q0 = GP_QUARTERS
                engs = ((nc.gpsimd, 0, q0), (nc.vector, q0, NSTRIP))
            elif COL_ENGINE == "gpsimd":
                engs = ((nc.gpsimd, 0, NSTRIP),)
            else:
                engs = ((nc.vector, 0, NSTRIP),)
            for eng, h0, h1 in engs:
                if h0 == h1:
                    continue
                sl = slice(h0 * SW, h1 * SW)
                if it == 0:
                    eng.tensor_copy(colmin[:, sl], cp[:, sl])
                else:
                    eng.tensor_tensor(colmin[:, sl], colmin[:, sl],
                                      cp[:, sl], amin)

            # row-min (free-dim)
            if ROW_MODE == "direct":
                nc.vector.tensor_reduce(rowacc[:, it:it + 1], cp[:],
                                        axis=mybir.AxisListType.X, op=amin)
            else:
                t1 = tmps.tile([128, SW], dt.bfloat16, tag="t1", name="t1")
                t2 = tmps.tile([128, SW], dt.bfloat16, tag="t2", name="t2")
                t3 = tmps.tile([128, SW], dt.bfloat16, tag="t3", name="t3")
                nc.vector.tensor_tensor(t1[:], cp[:, 0:SW],
                                        cp[:, SW:2 * SW], amin)
                nc.vector.tensor_tensor(t2[:], cp[:, 2 * SW:3 * SW],
                                        cp[:, 3 * SW:4 * SW], amin)
                nc.vector.tensor_tensor(t3[:], t1[:], t2[:], amin)
                nc.vector.tensor_reduce(rowacc[:, it:it + 1], t3[:],
                                        axis=mybir.AxisListType.X, op=amin)


def get_nc():
    if "nc" not in _CACHE:
        _CACHE["nc"] = _build_nc()
    return _CACHE["nc"]


def _make_runner(nc):
    """Build a cached jitted SPMD callable for `nc` (one NEFF on all 8
    cores, per-core inputs sharded along axis 0)."""
    import jax
    from jax.sharding import Mesh, PartitionSpec
    from jax.experimental.shard_map import shard_map
    from concourse.bass2jax import (
        _bass_exec_p,
        install_neuronx_cc_hook,
        partition_id_tensor,
    )

    install_neuronx_cc_hook()
    partition_name = (nc.partition_id_tensor.name
                      if nc.partition_id_tensor else None)

    in_names = []
    out_names = []
    out_avals = []
    out_shapes = []
    for alloc in nc.m.functions[0].allocations:
        if not isinstance(alloc, mybir.MemoryLocationSet):
            continue
        name = alloc.memorylocations[0].name
        if alloc.kind == "ExternalInput":
            if name != partition_name:
                in_names.append(name)
        elif alloc.kind == "ExternalOutput":
            shape = tuple(alloc.tensor_shape)
            dtype = mybir.dt.np(alloc.dtype)
            out_avals.append(jax.core.ShapedArray(shape, dtype))
            out_names.append(name)
            out_shapes.append((shape, dtype))
    n_params = len(in_names)
    n_outs = len(out_names)
    all_names = list(in_names) + list(out_names)
    if partition_name is not None:
        all_names.append(partition_name)
    donate = tuple(range(n_params, n_params + n_outs))

    def _body(*args):
        operands = list(args)
        if partition_name is not None:
            operands.append(partition_id_tensor())
        outs = _bass_exec_p.bind(
            *operands,
            out_avals=tuple(out_avals),
            in_names=tuple(all_names),
            out_names=tuple(out_names),
            lowering_input_output_aliases=(),
            sim_require_finite=True,
            sim_require_nnan=True,
            nc=nc,
        )
        return tuple(outs)

    devices = jax.devices()[:NCORES]
    mesh = Mesh(np.asarray(devices), ("core",))
    sharded = jax.jit(
        shard_map(_body, mesh=mesh,
                  in_specs=(PartitionSpec("core"),) * (n_params + n_outs),
                  out_specs=(PartitionSpec("core"),) * n_outs,
                  check_rep=False),
        donate_argnums=donate,
        keep_unused=True,
    )

    def prep(in_maps):
        concat_in = [
            np.concatenate([np.asarray(m[name]) for m in in_maps], axis=0)
            for name in in_names
        ]
        return concat_in

    def exec_prepped(concat_in):
        concat_zeros = [
            np.zeros((NCORES * s[0], *s[1:]), dt) for s, dt in out_shapes
        ]
        return sharded(*concat_in, *concat_zeros)

    def unpack(out_arrs):
        return [
            {
                name: np.asarray(out_arrs[i]).reshape(
                    NCORES, *out_shapes[i][0])[c]
                for i, name in enumerate(out_names)
            }
            for c in range(NCORES)
        ]

    def run(in_maps):
        return unpack(exec_prepped(prep(in_maps)))

    run.prep = prep
    run.exec_prepped = exec_prepped
    run.unpack = unpack
    run.mesh = mesh
    return run


def get_runner():
    if "run" not in _CACHE:
        _CACHE["run"] = _make_runner(get_nc())
    return _CACHE["run"]


def _f32(v):
    return np.asarray(v, dtype=np.float32)


def _bf(v):
    return np.asarray(v, dtype=np.float32).astype(bfloat16)


def build_rows(xc, yc):
    """Build the two [13, 4096] bf16 row tensors for one batch element.

    Contraction layout (k : L-row      * R-row):
      0-2 : -2*xh_d  * yh_d
      3-5 : -2*xl_d  * yh_d
      6-8 : -2*xh_d  * yl_d
      9   : sqx_h    * 1
      10  : sqx_l    * 1
      11  : 1        * sqy_h
      12  : 1        * sqy_l
    """
    def side(v):
        vh = _bf(v)
        vl = _bf(_f32(v) - _f32(vh))
        sq = (np.asarray(v, np.float64) ** 2).sum(-1)
        sqh = _bf(sq)
        sql = _bf(sq - np.float64(1.0) * _f32(sqh).astype(np.float64))
        m2h = _bf(-2.0 * _f32(vh))
        m2l = _bf(-2.0 * _f32(vl))
        return vh, vl, sqh, sql, m2h, m2l

    xh, xl, sqxh, sqxl, m2xh, m2xl = side(xc)
    yh, yl, sqyh, sqyl, m2yh, m2yl = side(yc)
    ones = np.ones((N,), dtype=bfloat16)

    la = np.stack([m2xh[:, 0], m2xh[:, 1], m2xh[:, 2],
                   m2xl[:, 0], m2xl[:, 1], m2xl[:, 2],
                   m2xh[:, 0], m2xh[:, 1], m2xh[:, 2],
                   sqxh, sqxl, ones, ones])
    ra = np.stack([yh[:, 0], yh[:, 1], yh[:, 2],
                   yh[:, 0], yh[:, 1], yh[:, 2],
                   yl[:, 0], yl[:, 1], yl[:, 2],
                   ones, ones, sqyh, sqyl])
    return {
        "la": np.ascontiguousarray(la),
        "ra": np.ascontiguousarray(ra),
    }


def _mins_to_vec(m):
    # m[p, it] is the min for point index it*128 + p
    return np.asarray(m, np.float64).T.reshape(N)


def kernel(x, y, x_mask, y_mask):
    x = np.asarray(x)
    y = np.asarray(y)
    in_maps = [build_rows(x[c], y[c]) for c in range(B)]
    res = get_runner()(in_maps)

    sa = 0.0
    sb = 0.0
    for c in range(B):
        minsA = _mins_to_vec(res[c]["minsA"])  # min over j, per x-point i
        minsB = _mins_to_vec(res[c]["minsB"])  # min over i, per y-point j
        sa += (np.asarray(x_mask[c], np.float64) * minsB).sum()
        sb += (np.asarray(y_mask[c], np.float64) * minsA).sum()
    a = sa / (B * N)
    b = sb / (B * N)
    return np.asarray((a - b) ** 2, dtype=np.float32)
